# revision 1
# baseline (speedup 1.0000x reference)
"""GroupedQueryAttention TP kernel for 8 Trainium2 NeuronCores.

Problem (hardcoded from the reference):
  B=2, S=2048, E=2048, H=32 q-heads, KV=8 kv-heads, D=128, fp32 I/O.
  y = GQA(x) with QK-RMSNorm, RoPE, causal mask, out-proj.

Sharding: data-parallel over batch (2) x tensor-parallel over heads (4).
  core c: batch b=c//4, tp-rank r=c%4 -> 8 q-heads, 2 kv-groups.
  Wq/Wk/Wv column-sharded, Wo row-sharded; partial outputs reduced
  across the 4 tp-ranks of each batch group.

Per-core kernel layout choices:
  - x^T blocks produced by DMA-transpose (bf16), consumed per s-chunk.
  - Q/K proj -> [s,d] fp32 -> RMSNorm+RoPE -> bf16 -> DMA-transpose to
    Q^T/K^T [d, s] (exactly the matmul operand layouts for scores).
  - scores computed transposed: scoresT[sk, sq] = K_tile @ Q^T, so
    ctx^T accumulates with lhsT = V[s,d] (no V transpose), and ctx^T
    [d, sq] is exactly the lhsT layout the out-proj needs.
  - softmax without max-subtraction (RMSNorm bounds |score| <= sqrt(D)),
    exp on ACT (PSUM->SBUF bf16), denominator via ones-matmul into PSUM,
    causal handled by loop bounds + affine_select on diagonal blocks.
"""

import math
import sys

sys.path.insert(0, "/opt/trn_rl_repo")

import numpy as np
import ml_dtypes

import concourse.bass as bass
import concourse.tile as tile
from concourse import mybir
from concourse.bass_utils import run_bass_kernel_spmd
from concourse.vector_clock import ScopedClock


def _install_ntff_hook_shim():
    """The agent image ships antenv without axon_hooks; recreate it so
    trace=True can capture NTFF profiles through libaxon_pjrt.so."""
    import types
    import ctypes
    import contextlib

    try:
        import antenv.axon_hooks  # noqa: F401
        return
    except ImportError:
        pass

    mod = types.ModuleType("antenv.axon_hooks")

    def _make_hook(so_path="/opt/axon/libaxon_pjrt.so"):
        try:
            lib = ctypes.CDLL(so_path)
        except OSError:
            return None
        if not hasattr(lib, "axon_start_nrt_profile"):
            return None
        lib.axon_start_nrt_profile.argtypes = [
            ctypes.POINTER(ctypes.c_int64),
            ctypes.c_size_t,
        ]
        lib.axon_start_nrt_profile.restype = ctypes.c_int64
        lib.axon_stop_nrt_profile.argtypes = [ctypes.c_char_p]
        lib.axon_stop_nrt_profile.restype = ctypes.c_int64

        @contextlib.contextmanager
        def _hook(output_dir, device_ids):
            import jax

            jax.devices()
            if device_ids:
                ids = (ctypes.c_int64 * len(device_ids))(*device_ids)
                rc = lib.axon_start_nrt_profile(ids, len(device_ids))
            else:
                rc = lib.axon_start_nrt_profile(None, 0)
            if rc != 0:
                raise RuntimeError(f"axon_start_nrt_profile rc={rc}")
            try:
                yield
            finally:
                n = lib.axon_stop_nrt_profile(str(output_dir).encode())
                if n < 0:
                    raise RuntimeError(f"axon_stop_nrt_profile rc={n}")

        return _hook

    _state = {}

    def get_axon_ntff_profile_hook():
        if "h" not in _state:
            _state["h"] = _make_hook()
        return _state["h"]

    def set_axon_ntff_profile_hook(hook):
        _state["h"] = hook

    mod.get_axon_ntff_profile_hook = get_axon_ntff_profile_hook
    mod.set_axon_ntff_profile_hook = set_axon_ntff_profile_hook
    sys.modules["antenv.axon_hooks"] = mod


_install_ntff_hook_shim()

F32 = mybir.dt.float32
BF16 = mybir.dt.bfloat16
AF = mybir.ActivationFunctionType
ALU = mybir.AluOpType

B, S, E = 2, 2048, 2048
H, KV, D = 32, 8, 128
TP = 4
HPC = H // TP          # 8 q-heads per core
G = KV // TP           # 2 kv-groups per core
SC = S // 128          # 16 s-chunks
ECH = E // 128         # 16 e-chunks
DQ = HPC * D           # 1024 q-proj cols per core
DKV = G * D            # 256 k (and v) proj cols per core
EPS = 1e-6
INV_SQRT_D = 1.0 / math.sqrt(D)

# ---------------------------------------------------------------------------
# Compat: this container's walrus codegen rejects >1 semaphore wait per
# instruction ("Too many sync wait commands").  Split extra waits onto
# preceding same-engine InstNoOp carriers.
# ---------------------------------------------------------------------------
MAXW = 1


def _split_waits_in_block_lists(nc, ordered):
    for _bb, insts in ordered.items():
        new_list = []
        for inst in insts:
            si = inst.sync_info
            if si is not None and len(si.on_wait) > MAXW:
                waits = list(si.on_wait)
                extra, keep = waits[:-MAXW], waits[-MAXW:]
                for i in range(0, len(extra), MAXW):
                    nop = mybir.InstNoOp(
                        name=nc.get_next_instruction_name(),
                        engine=inst.engine,
                        bass_nofuse=True,
                        sync_info=mybir.SyncInfo(
                            on_wait=extra[i : i + MAXW], on_update=[]
                        ),
                    )
                    new_list.append(nop)
                si.on_wait = keep
            new_list.append(inst)
        insts[:] = new_list


class CompatTileContext(tile.TileContext):
    @property
    def ordered_instructions_by_block(self):
        return self.__dict__.get("_ordered_instructions_by_block")

    @ordered_instructions_by_block.setter
    def ordered_instructions_by_block(self, value):
        if isinstance(value, dict):
            _split_waits_in_block_lists(self.nc, value)
        self.__dict__["_ordered_instructions_by_block"] = value

    def _drain_and_barrier(self, tick_clock, wait_clock):
        nc = self.nc
        probe = nc.sync.nop(nofuse=True)
        wait_clock.add_sem_waits(
            probe.ins, ScopedClock({None: tick_clock.global_clock})
        )
        si = probe.ins.sync_info
        waits = list(si.on_wait) if si is not None else []
        if len(waits) > MAXW:
            si.on_wait = waits[:MAXW]
            for i in range(MAXW, len(waits), MAXW):
                n2 = nc.sync.nop(nofuse=True)
                n2.ins.sync_info = mybir.SyncInfo(
                    on_wait=waits[i : i + MAXW], on_update=[]
                )
        nc.sync.drain()
        nc.all_engine_barrier()
        assert self.sems is not None
        popped = nc._tile_sem_poison_stack.pop()
        assert popped is self._sem_poison
        nc.clear_and_free_semaphores(list(self.sems.allocated().values()))
        nc.all_engine_barrier()


# ---------------------------------------------------------------------------
# Kernel builder
# ---------------------------------------------------------------------------


def build_kernel():
    nc = bass.Bass(
        "TRN2", target_bir_lowering=False, debug=False, num_devices=8
    )

    x_bf = nc.declare_dram_parameter("x_bf", [S, E], BF16, isOutput=False)
    wq = nc.declare_dram_parameter("wq", [E, DQ], BF16, isOutput=False)
    wkv = nc.declare_dram_parameter("wkv", [E, 2 * DKV], BF16, isOutput=False)
    wo = nc.declare_dram_parameter("wo", [DQ, E], BF16, isOutput=False)
    bq_d = nc.declare_dram_parameter("bq", [1, DQ], F32, isOutput=False)
    bkv_d = nc.declare_dram_parameter("bkv", [1, 2 * DKV], F32, isOutput=False)
    qs_d = nc.declare_dram_parameter("qs", [1, D], F32, isOutput=False)
    ks_d = nc.declare_dram_parameter("ks", [1, D], F32, isOutput=False)
    cos_d = nc.declare_dram_parameter("cos", [S, D], F32, isOutput=False)
    sin_d = nc.declare_dram_parameter("sin", [S, D], F32, isOutput=False)
    out_d = nc.declare_dram_parameter("out", [S, E], F32, isOutput=True)

    with CompatTileContext(nc) as tc:
        _emit(nc, tc, x_bf, wq, wkv, wo, bq_d, bkv_d, qs_d, ks_d, cos_d, sin_d, out_d)
    return nc


def _emit(nc, tc, x_bf, wq, wkv, wo, bq_d, bkv_d, qs_d, ks_d, cos_d, sin_d, out_d):
    from contextlib import ExitStack

    ctx = ExitStack()
    with ctx:
        # ---- persistent tensors -------------------------------------------
        persist = ctx.enter_context(tc.tile_pool(name="persist", bufs=1))
        qt_all = persist.tile([128, HPC, S], BF16, tag="qt_all")    # Q^T per head [d, s]
        kt_all = persist.tile([128, G, S], BF16, tag="kt_all")      # K^T per group [d, s]
        v_all = persist.tile([128, G, SC, D], BF16, tag="v_all")    # V per group [s, d] chunks
        ctxt_all = persist.tile([128, HPC, S], BF16, tag="ctxt_all")  # ctx^T per head [d, s]
        bq_bc = persist.tile([128, DQ], F32, tag="bq_bc")
        bkv_bc = persist.tile([128, 2 * DKV], F32, tag="bkv_bc")
        qs_bc = persist.tile([128, D], F32, tag="qs_bc")
        ks_bc = persist.tile([128, D], F32, tag="ks_bc")
        qs_rot = persist.tile([128, D], F32, tag="qs_rot")
        ks_rot = persist.tile([128, D], F32, tag="ks_rot")
        ones_bf = persist.tile([128, 1], BF16, tag="ones_bf")
        eps_t = persist.tile([128, 1], F32, tag="eps_t")
        nc.vector.memset(eps_t[:, :], EPS)

        nc.gpsimd.dma_start(out=bq_bc[:, :], in_=bq_d[:, :].to_broadcast((128, DQ)))
        nc.gpsimd.dma_start(out=bkv_bc[:, :], in_=bkv_d[:, :].to_broadcast((128, 2 * DKV)))
        nc.gpsimd.dma_start(out=qs_bc[:, :], in_=qs_d[:, :].to_broadcast((128, D)))
        nc.gpsimd.dma_start(out=ks_bc[:, :], in_=ks_d[:, :].to_broadcast((128, D)))
        # rotated scales: rope's sin term multiplies the rotated vector, whose
        # element d came from position (d+64)%128 - so fold the norm scale in
        # rotated order.
        h = D // 2
        nc.vector.tensor_copy(out=qs_rot[:, 0:h], in_=qs_bc[:, h:D])
        nc.vector.tensor_copy(out=qs_rot[:, h:D], in_=qs_bc[:, 0:h])
        nc.vector.tensor_copy(out=ks_rot[:, 0:h], in_=ks_bc[:, h:D])
        nc.vector.tensor_copy(out=ks_rot[:, h:D], in_=ks_bc[:, 0:h])
        nc.vector.memset(ones_bf[:, :], 1.0)

        # ==================================================================
        # Phase A: projections + norm + rope, build Q^T, K^T, V
        # ==================================================================
        ctx_a = ctx.enter_context(ExitStack())
        pool_a = ctx_a.enter_context(tc.tile_pool(name="pool_a", bufs=1))
        wq_sb = pool_a.tile([128, ECH, DQ], BF16, tag="wq_sb")
        wkv_sb = pool_a.tile([128, ECH, 2 * DKV], BF16, tag="wkv_sb")
        for ec in range(ECH):
            nc.sync.dma_start(out=wq_sb[:, ec, :], in_=wq[ec * 128 : (ec + 1) * 128, :])
            nc.sync.dma_start(out=wkv_sb[:, ec, :], in_=wkv[ec * 128 : (ec + 1) * 128, :])

        xt_pool = ctx_a.enter_context(tc.tile_pool(name="xt", bufs=2))
        psA = ctx_a.enter_context(tc.tile_pool(name="psA", bufs=3, space="PSUM"))
        qsb_pool = ctx_a.enter_context(tc.tile_pool(name="qsb", bufs=2))
        cs_pool = ctx_a.enter_context(tc.tile_pool(name="cs", bufs=2))
        tmp_pool = ctx_a.enter_context(tc.tile_pool(name="tmpA", bufs=4))
        stat_pool = ctx_a.enter_context(tc.tile_pool(name="stat", bufs=8))
        rope_out = ctx_a.enter_context(tc.tile_pool(name="rope_out", bufs=4))

        def norm_rope(src, scale_bc, scale_rot, cq, sq_, dst):
            """src: [128,128] f32 (s,d); writes bf16 transpose into dst [d,s]."""
            sq2 = tmp_pool.tile([128, D], F32, tag="sq2")
            ssum = stat_pool.tile([128, 1], F32, tag="ssum")
            nc.scalar.activation(
                out=sq2[:, :], in_=src, func=AF.Square, accum_out=ssum[:, :],
            )
            rstd = stat_pool.tile([128, 1], F32, tag="rstd")
            nc.scalar.activation(
                out=rstd[:, :], in_=ssum[:, :], func=AF.Sqrt,
                bias=eps_t[:, :], scale=1.0 / D,
            )
            nc.vector.reciprocal(out=rstd[:, :], in_=rstd[:, :])
            # t1 = q * rstd * (scale*cos);  t2 = rot(q) * rstd * (scale_rot*sin)
            t1 = tmp_pool.tile([128, D], F32, tag="t1")
            nc.vector.scalar_tensor_tensor(
                out=t1[:, :], in0=src, scalar=rstd[:, :], in1=cq[:, :],
                op0=ALU.mult, op1=ALU.mult,
            )
            rot = tmp_pool.tile([128, D], F32, tag="rot")
            nc.vector.tensor_scalar_mul(rot[:, 0:h], src[:, h:D], -1.0)
            nc.vector.tensor_copy(out=rot[:, h:D], in_=src[:, 0:h])
            t2 = tmp_pool.tile([128, D], F32, tag="t2")
            nc.vector.scalar_tensor_tensor(
                out=t2[:, :], in0=rot[:, :], scalar=rstd[:, :], in1=sq_[:, :],
                op0=ALU.mult, op1=ALU.mult,
            )
            qr = rope_out.tile([128, D], BF16, tag="qr")
            nc.vector.tensor_tensor(out=qr[:, :], in0=t1[:, :], in1=t2[:, :], op=ALU.add)
            nc.scalar.dma_start_transpose(out=dst, in_=qr[:, :])

        for sc in range(SC):
            s0 = sc * 128
            xt = xt_pool.tile([128, ECH, 128], BF16, tag="xt")
            for ec in range(ECH):
                nc.scalar.dma_start_transpose(
                    out=xt[:, ec, :],
                    in_=x_bf[s0 : s0 + 128, ec * 128 : (ec + 1) * 128],
                )
            # cos/sin slices for this s-chunk, premultiplied by norm scales
            cos_sc = cs_pool.tile([128, D], F32, tag="cos_sc")
            sin_sc = cs_pool.tile([128, D], F32, tag="sin_sc")
            nc.sync.dma_start(out=cos_sc[:, :], in_=cos_d[s0 : s0 + 128, :])
            nc.sync.dma_start(out=sin_sc[:, :], in_=sin_d[s0 : s0 + 128, :])
            cos_q = cs_pool.tile([128, D], F32, tag="cos_q")
            sin_q = cs_pool.tile([128, D], F32, tag="sin_q")
            cos_k = cs_pool.tile([128, D], F32, tag="cos_k")
            sin_k = cs_pool.tile([128, D], F32, tag="sin_k")
            nc.vector.tensor_tensor(out=cos_q[:, :], in0=cos_sc[:, :], in1=qs_bc[:, :], op=ALU.mult)
            nc.vector.tensor_tensor(out=sin_q[:, :], in0=sin_sc[:, :], in1=qs_rot[:, :], op=ALU.mult)
            nc.vector.tensor_tensor(out=cos_k[:, :], in0=cos_sc[:, :], in1=ks_bc[:, :], op=ALU.mult)
            nc.vector.tensor_tensor(out=sin_k[:, :], in0=sin_sc[:, :], in1=ks_rot[:, :], op=ALU.mult)

            # --- Q projection (1024 cols in two 512 psum tiles) ---
            q_sc = qsb_pool.tile([128, DQ], F32, tag="q_sc")
            for hf in range(2):
                pq = psA.tile([128, 512], F32, tag="pA")
                for ec in range(ECH):
                    nc.tensor.matmul(
                        pq[:, :], lhsT=xt[:, ec, :],
                        rhs=wq_sb[:, ec, hf * 512 : (hf + 1) * 512],
                        start=(ec == 0), stop=(ec == ECH - 1),
                    )
                nc.vector.scalar_tensor_tensor(
                    out=q_sc[:, hf * 512 : (hf + 1) * 512], in0=pq[:, :],
                    scalar=1.0, in1=bq_bc[:, hf * 512 : (hf + 1) * 512],
                    op0=ALU.mult, op1=ALU.add,
                )
            # --- K|V projection (512 cols) ---
            pkv = psA.tile([128, 512], F32, tag="pA")
            for ec in range(ECH):
                nc.tensor.matmul(
                    pkv[:, :], lhsT=xt[:, ec, :], rhs=wkv_sb[:, ec, :],
                    start=(ec == 0), stop=(ec == ECH - 1),
                )
            k_sc = qsb_pool.tile([128, DKV], F32, tag="k_sc")
            nc.vector.scalar_tensor_tensor(
                out=k_sc[:, :], in0=pkv[:, 0:DKV], scalar=1.0,
                in1=bkv_bc[:, 0:DKV], op0=ALU.mult, op1=ALU.add,
            )
            # V: bias-add + bf16 cast straight from PSUM
            for g in range(G):
                nc.vector.scalar_tensor_tensor(
                    out=v_all[:, g, sc, :], in0=pkv[:, DKV + g * D : DKV + (g + 1) * D],
                    scalar=1.0, in1=bkv_bc[:, DKV + g * D : DKV + (g + 1) * D],
                    op0=ALU.mult, op1=ALU.add,
                )
            # --- norm + rope + transpose ---
            for hh in range(HPC):
                norm_rope(
                    q_sc[:, hh * D : (hh + 1) * D], qs_bc, qs_rot,
                    cos_q, sin_q, qt_all[:, hh, s0 : s0 + 128],
                )
            for g in range(G):
                norm_rope(
                    k_sc[:, g * D : (g + 1) * D], ks_bc, ks_rot,
                    cos_k, sin_k, kt_all[:, g, s0 : s0 + 128],
                )

        ctx_a.close()

        # ==================================================================
        # Phase B: attention (transposed scores) + out-projection
        # ==================================================================
        pool_b = ctx.enter_context(tc.tile_pool(name="pool_b", bufs=1))
        wo_sb = pool_b.tile([128, HPC, E], BF16, tag="wo_sb")
        for hc in range(HPC):
            nc.sync.dma_start(out=wo_sb[:, hc, :], in_=wo[hc * 128 : (hc + 1) * 128, :])

        ps_pool = ctx.enter_context(tc.tile_pool(name="ps_s", bufs=2, space="PSUM"))
        pctx_pool = ctx.enter_context(tc.tile_pool(name="ps_ctx", bufs=2, space="PSUM"))
        pden_pool = ctx.enter_context(tc.tile_pool(name="ps_den", bufs=2, space="PSUM"))
        po_pool = ctx.enter_context(tc.tile_pool(name="ps_o", bufs=2, space="PSUM"))
        probs_pool = ctx.enter_context(tc.tile_pool(name="probs", bufs=4))
        den_pool = ctx.enter_context(tc.tile_pool(name="den", bufs=4))
        dram_pool = ctx.enter_context(tc.tile_pool(name="dscr", bufs=4, space="DRAM"))
        osb_pool = ctx.enter_context(tc.tile_pool(name="osb", bufs=3))

        for cq in range(4):          # 512-wide sq chunk of the output rows
            q0 = cq * 512
            n_skc = 4 * cq + 4       # causal: sk chunks 0 .. 4cq+3
            for hh in range(HPC):
                g = hh // (HPC // G)
                pctx = pctx_pool.tile([128, 512], F32, tag="pctx")
                pden = pden_pool.tile([1, 512], F32, tag="pden")
                for skc in range(n_skc):
                    ps = ps_pool.tile([128, 512], F32, tag="ps")
                    nc.tensor.matmul(
                        ps[:, :],
                        lhsT=kt_all[:, g, skc * 128 : (skc + 1) * 128],
                        rhs=qt_all[:, hh, q0 : q0 + 512],
                        start=True, stop=True,
                    )
                    probs = probs_pool.tile([128, 512], BF16, tag="probs")
                    nc.scalar.activation(
                        out=probs[:, :], in_=ps[:, :], func=AF.Exp,
                        scale=INV_SQRT_D,
                    )
                    if skc >= 4 * cq:
                        # diagonal block: keep sq >= sk, i.e.
                        # (q0 + j) - (skc*128 + p) >= 0
                        nc.gpsimd.affine_select(
                            out=probs[:, :], in_=probs[:, :],
                            compare_op=ALU.is_ge, fill=0.0,
                            base=q0 - skc * 128,
                            pattern=[[1, 512]], channel_multiplier=-1,
                        )
                    nc.tensor.matmul(
                        pctx[:, :], lhsT=v_all[:, g, skc, :], rhs=probs[:, :],
                        start=(skc == 0), stop=(skc == n_skc - 1),
                    )
                    nc.tensor.matmul(
                        pden[:, :], lhsT=ones_bf[:, :], rhs=probs[:, :],
                        start=(skc == 0), stop=(skc == n_skc - 1),
                    )
                # normalize: ctx^T[:, sq] /= denom[sq]
                rden1 = den_pool.tile([1, 512], F32, tag="rden1")
                nc.vector.reciprocal(out=rden1[:, :], in_=pden[:, :])
                dscr = dram_pool.tile([1, 512], F32, tag="dscr")
                nc.sync.dma_start(out=dscr[:, :], in_=rden1[:, :])
                rden = den_pool.tile([128, 512], F32, tag="rden")
                nc.sync.dma_start(out=rden[:, :], in_=dscr[:, :].to_broadcast((128, 512)))
                nc.vector.tensor_tensor(
                    out=ctxt_all[:, hh, q0 : q0 + 512], in0=pctx[:, :],
                    in1=rden[:, :], op=ALU.mult,
                )
            # out-projection for the four 128-row chunks of this sq range
            for sqc in range(4 * cq, 4 * cq + 4):
                for oc in range(4):
                    po = po_pool.tile([128, 512], F32, tag="po")
                    for hc in range(HPC):
                        nc.tensor.matmul(
                            po[:, :],
                            lhsT=ctxt_all[:, hc, sqc * 128 : (sqc + 1) * 128],
                            rhs=wo_sb[:, hc, oc * 512 : (oc + 1) * 512],
                            start=(hc == 0), stop=(hc == HPC - 1),
                        )
                    osb = osb_pool.tile([128, 512], F32, tag="osb")
                    nc.scalar.copy(out=osb[:, :], in_=po[:, :])
                    nc.sync.dma_start(
                        out=out_d[sqc * 128 : (sqc + 1) * 128, oc * 512 : (oc + 1) * 512],
                        in_=osb[:, :],
                    )


_NC_CACHE = {}


def _get_nc():
    if "nc" not in _NC_CACHE:
        _NC_CACHE["nc"] = build_kernel()
    return _NC_CACHE["nc"]


def _shard_inputs(x, mask, cos, sin, Wq, bq, Wk, bk, Wv, bv, Wo, q_scale, k_scale):
    bf = ml_dtypes.bfloat16
    x_b = [np.ascontiguousarray(x[b]).astype(bf) for b in range(B)]
    cos32 = np.ascontiguousarray(cos.astype(np.float32))
    sin32 = np.ascontiguousarray(sin.astype(np.float32))
    qs = q_scale.reshape(1, D).astype(np.float32)
    ks = k_scale.reshape(1, D).astype(np.float32)
    in_maps = []
    for c in range(8):
        b, r = c // TP, c % TP
        wq_r = np.ascontiguousarray(Wq[:, r * DQ : (r + 1) * DQ]).astype(bf)
        wk_r = Wk[:, r * DKV : (r + 1) * DKV]
        wv_r = Wv[:, r * DKV : (r + 1) * DKV]
        wkv_r = np.ascontiguousarray(np.concatenate([wk_r, wv_r], axis=1)).astype(bf)
        wo_r = np.ascontiguousarray(Wo[r * DQ : (r + 1) * DQ, :]).astype(bf)
        bq_r = np.ascontiguousarray(bq[r * DQ : (r + 1) * DQ]).reshape(1, DQ).astype(np.float32)
        bkv_r = np.concatenate(
            [bk[r * DKV : (r + 1) * DKV], bv[r * DKV : (r + 1) * DKV]]
        ).reshape(1, 2 * DKV).astype(np.float32)
        in_maps.append(
            {
                "x_bf": x_b[b],
                "wq": wq_r,
                "wkv": wkv_r,
                "wo": wo_r,
                "bq": bq_r,
                "bkv": bkv_r,
                "qs": qs,
                "ks": ks,
                "cos": cos32,
                "sin": sin32,
            }
        )
    return in_maps


def kernel(x, mask, cos, sin, Wq, bq, Wk, bk, Wv, bv, Wo, q_scale, k_scale,
           _trace=False, _trace_kwargs=None):
    x = np.asarray(x, dtype=np.float32)
    in_maps = _shard_inputs(
        x, mask, np.asarray(cos), np.asarray(sin),
        np.asarray(Wq), np.asarray(bq), np.asarray(Wk), np.asarray(bk),
        np.asarray(Wv), np.asarray(bv), np.asarray(Wo),
        np.asarray(q_scale), np.asarray(k_scale),
    )
    nc = _get_nc()
    res = run_bass_kernel_spmd(
        nc, in_maps, list(range(8)), trace=_trace,
        **(_trace_kwargs or {}),
    )
    out = np.zeros((B, S, E), dtype=np.float32)
    for c in range(8):
        b = c // TP
        out[b] += res.results[c]["out"]
    if _trace:
        kernel._last_result = res
    return out

